# revision 1
# baseline (speedup 1.0000x reference)
"""Trainium2 Bass kernel for pre-LN multi-head self-attention.

Problem shapes (hardcoded): q (4, 2048, 1024) f32, attn_mask (2048, 2048) bool,
Wq/Wk/Wv (1024, 1024) f32, Wo (1024, 1024) f32, gamma/beta (1024,) f32.
N_HEAD=16, D_HEAD=64, pre-layernorm, softmax over the key axis.

Sharding: 8 cores = 4 batches x 2 head-groups (8 heads each). Each core
computes LN(q_b), its head-slice of the QKV projections, attention for its
8 heads, and a partial O-projection. The O-projection "all-reduce" over the
2 head-groups of a batch plus the qn residual add happens at host gather time.

On-device pipeline per core (all matmuls bf16 with fp32 PSUM accumulation):
  LN(f32, bn_stats)  ->  qn  -> PE-transpose -> qnT (bf16)
  qnT @ W{q,k}  -> hqT/hkT [head_dim, seq]   (per-head slices at partition
                                              offsets 0/64 -> PE row-tiling)
  qnT @ Wv      -> hv [seq, head_dim] augmented with a ones column so the
                   AV matmul also produces the softmax denominators
  S^T[j,i] = hkT_h^T' . hqT_h  (K=64 matmuls, two heads packed in the array)
  P = exp(SCALE*S) (ACT, fused scale) * mask01 (DVE, bf16 2x)
  vecT_aug = hv_aug^T' . P  (accumulated over j chunks; row 64 = denominators)
  vecT *= 1/denom  (reciprocal_approx + ones-column broadcast matmul)
  part = vecT^T' . Wo  -> DMA out (f32);  qn is DMA'd out separately (f32).
"""

import numpy as np
import ml_dtypes
from contextlib import ExitStack

import concourse.bass as bass
import concourse.tile as tile
from concourse import bacc, mybir
from concourse.bass_utils import run_bass_kernel_spmd

F32 = mybir.dt.float32
BF16 = mybir.dt.bfloat16
Alu = mybir.AluOpType
Act = mybir.ActivationFunctionType

BSZ, SEQ, DM = 4, 2048, 1024
NH, DH = 16, 64
HPC = 8              # heads per core
HD = HPC * DH        # 512 = per-core slice of the head dim
NCORES = 8
SCALE = 1.0 / (DH ** 0.5)
LN_EPS = 1e-5

NT = SEQ // 128      # 16 seq tiles of 128
NDC = DM // 128      # 8 d_model chunks of 128
NHC = HD // 128      # 4 per-core head-dim chunks of 128
HVW = DH + 1         # 65: hv columns per head incl. the ones column


def _mha_tile(ctx, tc, dq, dmask, dwq, dwk, dwv, dwo, dident, dgamma, dbeta,
              dqn, dpart, parts="full"):
    nc = tc.nc
    do_qkv = parts in ("qkv", "attn", "full")
    do_attn = parts in ("attn", "full")
    do_o = parts == "full"

    persist = ctx.enter_context(tc.tile_pool(name="persist", bufs=1))
    ident_sb = persist.tile([128, 128], BF16)
    nc.sync.dma_start(out=ident_sb, in_=dident)
    eps_sb = persist.tile([128, 1], F32)
    nc.vector.memset(eps_sb, LN_EPS)

    hqT = persist.tile([128, NHC * SEQ], BF16)      # [hd-chunk part, seq]
    hkT = persist.tile([128, NHC * SEQ], BF16)
    hv = persist.tile([128, NT * HPC * HVW], BF16)  # [j part, per-jc 8*65]
    vecT = persist.tile([128, NHC * SEQ], BF16)     # [hd-chunk part, i]
    wo_sb = persist.tile([128, NHC * DM], BF16)

    gamma_sb = beta_sb = None
    if dgamma is not None:
        gamma_sb = persist.tile([128, DM], F32)
        beta_sb = persist.tile([128, DM], F32)
        nc.sync.dma_start(out=gamma_sb, in_=bass.AP(
            tensor=dgamma.tensor, offset=dgamma.offset,
            ap=[[0, 128]] + list(dgamma.ap)))
        nc.sync.dma_start(out=beta_sb, in_=bass.AP(
            tensor=dbeta.tensor, offset=dbeta.offset,
            ap=[[0, 128]] + list(dbeta.ap)))

    # ---------------- Phase A: LN + transpose + QKV projections -------------
    with tc.tile_pool(name="phA", bufs=1) as pA, \
         tc.tile_pool(name="qtiles", bufs=3) as qpool, \
         tc.tile_pool(name="qnbf", bufs=2) as qnbfpool, \
         tc.tile_pool(name="stats", bufs=6) as spool, \
         tc.tile_pool(name="psT", bufs=2, space="PSUM") as psT, \
         tc.tile_pool(name="psQK", bufs=3, space="PSUM") as psQK:

        qnT = pA.tile([128, NDC * SEQ], BF16)       # [dm-chunk part, seq]
        wq_sb = pA.tile([128, NDC * HD], BF16)
        wk_sb = pA.tile([128, NDC * HD], BF16)
        wv_sb = pA.tile([128, NDC * HD], BF16)

        for tb in range(NT // 4):
            # one merged DMA loads 4 seq tiles; one merged DMA stores 4 qn tiles
            rows4 = slice(tb * 512, (tb + 1) * 512)
            qt = qpool.tile([128, 4, DM], F32, tag="qt")
            nc.sync.dma_start(
                out=qt, in_=dq[rows4, :].rearrange("(a p) m -> p a m", p=128))
            if tb == 0:
                # weights are first needed by QKV(tb=0) ~40us in; issuing them
                # after the first q block keeps LN off the critical path
                for w_sb, dw in ((wq_sb, dwq), (wk_sb, dwk), (wv_sb, dwv)):
                    nc.sync.dma_start(
                        out=w_sb.rearrange("p (dc hd) -> p dc hd", hd=HD),
                        in_=dw.rearrange("(dc p) hd -> p dc hd", p=128))
            qnf = qt        # LN is computed in place: q values are dead after stats
            for k in range(4):
                st = spool.tile([128, 2, 6], F32, tag="st")
                nc.vector.bn_stats(out=st[:, 0, :], in_=qt[:, k, 0:512])
                nc.vector.bn_stats(out=st[:, 1, :], in_=qt[:, k, 512:1024])
                mv = spool.tile([128, 2], F32, tag="mv")
                nc.vector.bn_aggr(out=mv, in_=st)
                # rstd = exp(-0.5*ln(var+eps)) keeps ACT in the exp/ln table set
                lnv = spool.tile([128, 1], F32, tag="lnv")
                nc.scalar.activation(out=lnv, in_=mv[:, 1:2], func=Act.Ln,
                                     bias=eps_sb, scale=1.0)
                rstd = spool.tile([128, 1], F32, tag="rstd")
                nc.scalar.activation(out=rstd, in_=lnv, func=Act.Exp, scale=-0.5)
                negmr = spool.tile([128, 1], F32, tag="negmr")
                nc.vector.tensor_tensor(out=negmr, in0=mv[:, 0:1], in1=rstd,
                                        op=Alu.mult)
                nc.vector.tensor_scalar_mul(negmr, negmr, -1.0)
                nc.vector.tensor_scalar(out=qnf[:, k, :], in0=qt[:, k, :],
                                        scalar1=rstd, scalar2=negmr,
                                        op0=Alu.mult, op1=Alu.add)
                if gamma_sb is not None:
                    nc.vector.tensor_tensor(out=qnf[:, k, :], in0=qnf[:, k, :],
                                            in1=gamma_sb, op=Alu.mult)
                    nc.vector.tensor_tensor(out=qnf[:, k, :], in0=qnf[:, k, :],
                                            in1=beta_sb, op=Alu.add)
            nc.gpsimd.dma_start(
                out=dqn[rows4, :].rearrange("(a p) m -> p a m", p=128), in_=qnf)
            qnbf = qnbfpool.tile([128, 4, DM], BF16, tag="qnbf")
            nc.vector.tensor_copy(out=qnbf, in_=qnf)
            for dc in range(NDC):
                pst = psT.tile([128, 512], BF16, tag="pst")
                for k in range(4):
                    nc.tensor.transpose(pst[:, k * 128:(k + 1) * 128],
                                        qnbf[:, k, dc * 128:(dc + 1) * 128],
                                        ident_sb)
                nc.scalar.copy(
                    out=qnT[:, dc * SEQ + tb * 512: dc * SEQ + (tb + 1) * 512],
                    in_=pst)

            # QKV for this seq block (sc == tb): overlaps the next block's LN
            if not do_qkv:
                continue
            sc = tb
            for w_sb, dstT in ((wq_sb, hqT), (wk_sb, hkT)):
                for hc in range(NHC):
                    ps = psQK.tile([128, 512], F32, tag="psqk")
                    for dc in range(NDC):
                        nc.tensor.matmul(
                            ps,
                            lhsT=w_sb[:, dc * HD + hc * 128: dc * HD + (hc + 1) * 128],
                            rhs=qnT[:, dc * SEQ + sc * 512: dc * SEQ + (sc + 1) * 512],
                            start=(dc == 0), stop=(dc == NDC - 1))
                    nc.vector.tensor_copy(
                        out=dstT[:, hc * SEQ + sc * 512: hc * SEQ + (sc + 1) * 512],
                        in_=ps)
            for jc in range(4 * tb, 4 * tb + 4):
                ps = psQK.tile([128, HD], F32, tag="psv")
                for dc in range(NDC):
                    nc.tensor.matmul(
                        ps,
                        lhsT=qnT[:, dc * SEQ + jc * 128: dc * SEQ + jc * 128 + 128],
                        rhs=wv_sb[:, dc * HD:(dc + 1) * HD],
                        start=(dc == 0), stop=(dc == NDC - 1))
                blk = hv[:, jc * HPC * HVW:(jc + 1) * HPC * HVW]
                blk3 = blk.rearrange("p (h x) -> p h x", x=HVW)
                nc.vector.tensor_copy(out=blk3[:, :, 0:DH],
                                      in_=ps.rearrange("p (h x) -> p h x", x=DH))
        hv4 = hv.rearrange("p (j h x) -> p j h x", h=HPC, x=HVW)
        nc.vector.memset(hv4[:, :, :, DH:HVW], 1.0)

    # ---------------- Phase B: attention (two heads packed per pass) --------
    drecip = nc.dram_tensor(f"recip_scratch{nc.next_id()}", [HPC, SEQ], F32).ap()
    with tc.tile_pool(name="mk", bufs=1) as mkpool, \
         tc.tile_pool(name="pp", bufs=3) as ppool, \
         tc.tile_pool(name="stg", bufs=1) as stpool, \
         tc.tile_pool(name="den", bufs=1) as denpool, \
         tc.tile_pool(name="sps", bufs=1, space="PSUM") as spsum, \
         tc.tile_pool(name="vps", bufs=1, space="PSUM") as vpsum:
        # whole mask resident: [j, i] in jc column blocks, 8 chunked DMAs
        mask_all = mkpool.tile([128, NT * SEQ], BF16)
        for c in range(8):
            nc.scalar.dma_start(
                out=mask_all[:, c * 2 * SEQ:(c + 1) * 2 * SEQ]
                    .rearrange("p (a i) -> p a i", i=SEQ),
                in_=dmask[c * 256:(c + 1) * 256, :]
                    .rearrange("(a p) i -> p a i", p=128))
        nc.scalar.dma_start(
            out=wo_sb.rearrange("p (hc m) -> p hc m", m=DM),
            in_=dwo.rearrange("(hc p) m -> p hc m", p=128))
        if not do_attn:
            nc.vector.memset(vecT, 0.0)
        for hp in range(HPC // 2 if do_attn else 0):
            ha, hb = 2 * hp, 2 * hp + 1
            den_hp = denpool.tile([2, SEQ], F32, tag="den")
            for ih in range(2):
                vA = vpsum.tile([65, 1024], F32, tag="vA")
                vB = vpsum.tile([65, 1024], F32, tag="vB")
                for jc in range(NT):
                    mk = mask_all[:, jc * SEQ + ih * 1024: jc * SEQ + (ih + 1) * 1024]
                    sA = spsum.tile([128, 1024], F32, tag="sA")
                    sB = spsum.tile([128, 1024], F32, tag="sB")
                    kslice = slice(hp * SEQ + jc * 128, hp * SEQ + (jc + 1) * 128)
                    for n in range(2):
                        qsl = slice(hp * SEQ + ih * 1024 + n * 512,
                                    hp * SEQ + ih * 1024 + (n + 1) * 512)
                        osl = slice(n * 512, (n + 1) * 512)
                        nc.tensor.matmul(sA[:, osl], lhsT=hkT[0:64, kslice],
                                         rhs=hqT[0:64, qsl], start=True, stop=True)
                        nc.tensor.matmul(sB[:, osl], lhsT=hkT[64:128, kslice],
                                         rhs=hqT[64:128, qsl], start=True, stop=True)
                    pa = ppool.tile([128, 1024], BF16, tag="pa")
                    pb = ppool.tile([128, 1024], BF16, tag="pb")
                    nc.scalar.activation(out=pa, in_=sA, func=Act.Exp, scale=SCALE)
                    nc.scalar.activation(out=pb, in_=sB, func=Act.Exp, scale=SCALE)
                    nc.vector.tensor_mul(pa, pa, mk)
                    nc.vector.tensor_mul(pb, pb, mk)
                    va_l = hv[:, jc * HPC * HVW + ha * HVW: jc * HPC * HVW + ha * HVW + HVW]
                    vb_l = hv[:, jc * HPC * HVW + hb * HVW: jc * HPC * HVW + hb * HVW + HVW]
                    for n in range(2):
                        osl = slice(n * 512, (n + 1) * 512)
                        nc.tensor.matmul(vA[:, osl], lhsT=va_l, rhs=pa[:, osl],
                                         start=(jc == 0), stop=(jc == NT - 1))
                        nc.tensor.matmul(vB[:, osl], lhsT=vb_l, rhs=pb[:, osl],
                                         start=(jc == 0), stop=(jc == NT - 1))
                isl = slice(hp * SEQ + ih * 1024, hp * SEQ + (ih + 1) * 1024)
                dsl = slice(ih * 1024, (ih + 1) * 1024)
                # denominators (psum row 64) are staged to SBUF (DMA cannot
                # read PSUM) then DMA'd to den_hp partitions 0/1; head-b vec
                # rows must move from psum partitions 0:64 to sbuf 64:128 --
                # both are cross-partition moves only a DMA can do.
                dsa = stpool.tile([65, 1024], F32, tag="dsa")
                dsb = stpool.tile([65, 1024], F32, tag="dsb")
                nc.vector.tensor_copy(out=dsa[64:65, :], in_=vA[64:65, :])
                nc.vector.tensor_copy(out=dsb[64:65, :], in_=vB[64:65, :])
                nc.sync.dma_start(out=den_hp[0:1, dsl], in_=dsa[64:65, :])
                nc.sync.dma_start(out=den_hp[1:2, dsl], in_=dsb[64:65, :])
                nc.vector.tensor_copy(out=vecT[0:64, isl], in_=vA[0:64, :])
                stage = stpool.tile([64, 1024], BF16, tag="stg")
                nc.vector.tensor_copy(out=stage, in_=vB[0:64, :])
                nc.gpsimd.dma_start(out=vecT[64:128, isl], in_=stage)
            # normalize this head pair while the next pair's attention runs
            recip_hp = denpool.tile([2, SEQ], F32, tag="recip")
            nc.vector.reciprocal_approx_fast(out=recip_hp, in_=den_hp)
            nc.sync.dma_start(out=drecip[ha:hb + 1, :], in_=recip_hp)
            bc_hp = denpool.tile([128, SEQ], F32, tag="bc")
            for h, lo in ((ha, 0), (hb, 64)):
                row = drecip[h:h + 1, :]
                nc.sync.dma_start(
                    out=bc_hp[lo:lo + 64, :],
                    in_=bass.AP(tensor=row.tensor, offset=row.offset,
                                ap=[[0, 64]] + list(row.ap[1:])))
            hsl = slice(hp * SEQ, (hp + 1) * SEQ)
            nc.vector.tensor_tensor(out=vecT[:, hsl], in0=vecT[:, hsl],
                                    in1=bc_hp, op=Alu.mult)

    # ---------------- Phase C: O-projection ---------------------------------
    with tc.tile_pool(name="po", bufs=4, space="PSUM") as opool, \
         tc.tile_pool(name="outs", bufs=2) as outpool:
        for ob in range(NT // 4):
            outt = outpool.tile([128, 4, DM], F32, tag="outt")
            if not do_o:
                nc.vector.memset(outt, 0.0)
                nc.sync.dma_start(
                    out=dpart[ob * 512:(ob + 1) * 512, :]
                        .rearrange("(a p) m -> p a m", p=128),
                    in_=outt)
                continue
            for k in range(4):
                it = ob * 4 + k
                for mc in range(2):
                    po = opool.tile([128, 512], F32, tag="po")
                    for hc in range(NHC):
                        nc.tensor.matmul(
                            po,
                            lhsT=vecT[:, hc * SEQ + it * 128: hc * SEQ + (it + 1) * 128],
                            rhs=wo_sb[:, hc * DM + mc * 512: hc * DM + (mc + 1) * 512],
                            start=(hc == 0), stop=(hc == NHC - 1))
                    nc.vector.tensor_copy(out=outt[:, k, mc * 512:(mc + 1) * 512],
                                          in_=po)
            nc.sync.dma_start(
                out=dpart[ob * 512:(ob + 1) * 512, :]
                    .rearrange("(a p) m -> p a m", p=128),
                in_=outt)


_NC_CACHE = {}


def _build(gamma_trivial, repeat=1, parts="full"):
    key = (bool(gamma_trivial), repeat, parts)
    if key in _NC_CACHE:
        return _NC_CACHE[key]
    nc = bacc.Bacc("TRN2", target_bir_lowering=False, debug=False,
                   num_devices=NCORES)
    dq = nc.dram_tensor("q", [SEQ, DM], F32, kind="ExternalInput").ap()
    dmask = nc.dram_tensor("maskt", [SEQ, SEQ], BF16, kind="ExternalInput").ap()
    dwq = nc.dram_tensor("wq", [DM, HD], BF16, kind="ExternalInput").ap()
    dwk = nc.dram_tensor("wk", [DM, HD], BF16, kind="ExternalInput").ap()
    dwv = nc.dram_tensor("wv", [DM, HD], BF16, kind="ExternalInput").ap()
    dwo = nc.dram_tensor("wo", [HD, DM], BF16, kind="ExternalInput").ap()
    dident = nc.dram_tensor("ident", [128, 128], BF16, kind="ExternalInput").ap()
    dgamma = dbeta = None
    if not gamma_trivial:
        dgamma = nc.dram_tensor("gamma", [DM], F32, kind="ExternalInput").ap()
        dbeta = nc.dram_tensor("beta", [DM], F32, kind="ExternalInput").ap()
    dqn = nc.dram_tensor("qn_out", [SEQ, DM], F32, kind="ExternalOutput").ap()
    dpart = nc.dram_tensor("part_out", [SEQ, DM], F32, kind="ExternalOutput").ap()
    with tile.TileContext(nc) as tc:
        for _rep in range(repeat):
            with ExitStack() as ctx:
                _mha_tile(ctx, tc, dq, dmask, dwq, dwk, dwv, dwo, dident,
                          dgamma, dbeta, dqn, dpart, parts=parts)
    nc.compile()
    _NC_CACHE[key] = nc
    return nc


def _run(nc, in_maps, **kwargs):
    return run_bass_kernel_spmd(nc, in_maps, list(range(NCORES)), **kwargs)


def make_in_maps(q, attn_mask, Wq, Wk, Wv, Wo, gamma, beta, gamma_trivial):
    bf = ml_dtypes.bfloat16
    q = np.ascontiguousarray(np.asarray(q, dtype=np.float32))
    maskt = np.ascontiguousarray(
        (~np.asarray(attn_mask, dtype=bool)).T.astype(bf))
    Wq = np.asarray(Wq, dtype=np.float32)
    Wk = np.asarray(Wk, dtype=np.float32)
    Wv = np.asarray(Wv, dtype=np.float32)
    Wo = np.asarray(Wo, dtype=np.float32)
    ident = np.eye(128, dtype=bf)
    in_maps = []
    for c in range(NCORES):
        b, g = c // 2, c % 2
        cols = slice(g * HD, (g + 1) * HD)
        m = {
            "q": q[b],
            "maskt": maskt,
            "wq": np.ascontiguousarray(Wq[:, cols].astype(bf)),
            "wk": np.ascontiguousarray(Wk[:, cols].astype(bf)),
            "wv": np.ascontiguousarray(Wv[:, cols].astype(bf)),
            "wo": np.ascontiguousarray(Wo[cols, :].astype(bf)),
            "ident": ident,
        }
        if not gamma_trivial:
            m["gamma"] = np.asarray(gamma, dtype=np.float32)
            m["beta"] = np.asarray(beta, dtype=np.float32)
        in_maps.append(m)
    return in_maps


def kernel(q, attn_mask, Wq, Wk, Wv, Wo, gamma, beta):
    gamma_np = np.asarray(gamma, dtype=np.float32)
    beta_np = np.asarray(beta, dtype=np.float32)
    gamma_trivial = bool(np.all(gamma_np == 1.0) and np.all(beta_np == 0.0))
    nc = _build(gamma_trivial)
    in_maps = make_in_maps(q, attn_mask, Wq, Wk, Wv, Wo, gamma_np, beta_np,
                           gamma_trivial)
    res = _run(nc, in_maps).results
    out = np.empty((BSZ, SEQ, DM), dtype=np.float32)
    for b in range(BSZ):
        out[b] = res[2 * b]["qn_out"]
        out[b] += res[2 * b]["part_out"]
        out[b] += res[2 * b + 1]["part_out"]
    return out


if __name__ == "__main__":
    rng = np.random.default_rng(0)
    ins = {
        "q": rng.standard_normal((BSZ, SEQ, DM), dtype=np.float32),
        "attn_mask": rng.integers(0, 2, (SEQ, SEQ)).astype(bool),
        "Wq": rng.standard_normal((DM, NH * DH), dtype=np.float32) * 0.03,
        "Wk": rng.standard_normal((DM, NH * DH), dtype=np.float32) * 0.03,
        "Wv": rng.standard_normal((DM, NH * DH), dtype=np.float32) * 0.03,
        "Wo": rng.standard_normal((NH * DH, DM), dtype=np.float32) * 0.03,
        "gamma": np.ones(DM, np.float32),
        "beta": np.zeros(DM, np.float32),
    }
    out = kernel(**ins)
    print("kernel ran, out shape", out.shape, out.dtype)



# revision 10
# speedup vs baseline: 1.0601x; 1.0601x over previous
"""Trainium2 Bass kernel for pre-LN multi-head self-attention.

Problem shapes (hardcoded): q (4, 2048, 1024) f32, attn_mask (2048, 2048) bool,
Wq/Wk/Wv (1024, 1024) f32, Wo (1024, 1024) f32, gamma/beta (1024,) f32.
N_HEAD=16, D_HEAD=64, pre-layernorm, softmax over the key axis.

Sharding: 8 cores = 4 batches x 2 head-groups (8 heads each). Each core
computes LN(q_b), its head-slice of the QKV projections, attention for its
8 heads, and a partial O-projection. The O-projection "all-reduce" over the
2 head-groups of a batch plus the qn residual add happens at host gather time.

Key structure (v2):
  - Host ships q twice: row-major (for LN stats + the qn residual output)
    and transposed (QKV rhs) -- no on-device transposes at all.
  - LayerNorm folds into the QKV matmuls: the -mu term is a K=1 matmul
    (lhsT = column-sums of W, rhs = -mu row), the rstd term multiplies the
    Q-side projection during the PSUM->SBUF move and rides the softmax-exp's
    per-partition scale operand on the K side; the V side is a per-partition
    tensor_scalar.
  - Phase B processes (head-pair, i-half, j-chunk) blocks: both heads'
    scores land in one bf16 PSUM tile [128, 2048] (1 bank per head,
    double-buffered), ONE exp covers both heads, and emission is software-
    pipelined (scores of block n+1 issue before the AV matmuls of block n)
    so the scalar engine's exp stream never starves behind PE sem-waits.
  - AV uses the ones-column trick (M=65) so softmax denominators come from
    the same PE stream as the weighted values.
"""

import numpy as np
import ml_dtypes
from contextlib import ExitStack

import concourse.bass as bass
import concourse.tile as tile
from concourse import bacc, mybir
from concourse.bass_utils import run_bass_kernel_spmd

F32 = mybir.dt.float32
BF16 = mybir.dt.bfloat16
Alu = mybir.AluOpType
Act = mybir.ActivationFunctionType

BSZ, SEQ, DM = 4, 2048, 1024
NH, DH = 16, 64
HPC = 8              # heads per core
HD = HPC * DH        # 512 = per-core slice of the head dim
NCORES = 8
SCALE = 1.0 / (DH ** 0.5)
LN_EPS = 1e-5

NT = SEQ // 128      # 16 seq tiles of 128
NDC = DM // 128      # 8 d_model chunks of 128
NHC = HD // 128      # 4 per-core head-dim chunks of 128
HVW = DH + 1         # 65: hv columns per head incl. the ones column


def _mha_tile(ctx, tc, dqbf, dqt, dmask, dwq, dwk, dwv, dwo, dwsums, dwbeta,
              dgammabf, dbetabf, dqn, dpart):
    nc = tc.nc
    trivial = dgammabf is None

    persist = ctx.enter_context(tc.tile_pool(name="persist", bufs=1))
    hqT = persist.tile([128, NHC * SEQ], BF16)      # [hd-chunk part, seq] (LN'd)
    hkT = persist.tile([128, NHC * SEQ], BF16)      # [hd-chunk part, seq] (no rstd_j)
    hv = persist.tile([128, NT * HPC * HVW], BF16)  # [j part, per-jc 8*65]
    vecT = persist.tile([128, NHC * SEQ], BF16)     # [hd-chunk part, i]
    wo_sb = persist.tile([128, NHC * DM], BF16)
    rstd_bc = persist.tile([128, SEQ], BF16)        # rstd[i] on all partitions
    nmrow = persist.tile([1, SEQ], BF16)            # -mu[i] as a row
    # rstd*SCALE per (j-tile) column: per-partition scale for the softmax exp
    rstd_sc = persist.tile([128, NT], F32)
    ones_row = None
    if not trivial:
        ones_row = persist.tile([1, SEQ], BF16)
        nc.vector.memset(ones_row, 1.0)

    # per tb: rows 0:4 = -mu (k-major), rows 4:8 = rstd; contiguous rows so
    # the row-gather and partition-broadcast read-backs are simple 2D APs
    dstat = nc.dram_tensor(f"stat_scratch{nc.next_id()}", [32, 128], BF16).ap()

    # ---------------- Phase A: LN stats + folded QKV projections ------------
    with tc.tile_pool(name="phA", bufs=1) as pA, \
         tc.tile_pool(name="qtiles", bufs=2) as qpool, \
         tc.tile_pool(name="stats", bufs=4) as spool, \
         tc.tile_pool(name="rowbf", bufs=2) as rpool, \
         tc.tile_pool(name="psQK", bufs=3, space="PSUM") as psQK:

        qT_sb = pA.tile([128, NDC * SEQ], BF16)     # [dm-chunk part, seq]
        wq_sb = pA.tile([128, NDC * HD], BF16)
        wk_sb = pA.tile([128, NDC * HD], BF16)
        wv_sb = pA.tile([128, NDC * HD], BF16)
        wsums_sb = pA.tile([1, 3 * HD], BF16)
        wbeta_sb = None
        gbc = bbc = None
        if not trivial:
            wbeta_sb = pA.tile([1, 3 * HD], BF16)
            nc.scalar.dma_start(out=wbeta_sb, in_=dwbeta)
            gbc = pA.tile([128, DM], BF16)
            bbc = pA.tile([128, DM], BF16)
            nc.scalar.dma_start(out=gbc, in_=bass.AP(
                tensor=dgammabf.tensor, offset=dgammabf.offset,
                ap=[[0, 128]] + list(dgammabf.ap)))
            nc.scalar.dma_start(out=bbc, in_=bass.AP(
                tensor=dbetabf.tensor, offset=dbetabf.offset,
                ap=[[0, 128]] + list(dbetabf.ap)))

        nc.scalar.dma_start(
            out=qT_sb.rearrange("p (dc s) -> p dc s", s=SEQ),
            in_=dqt.rearrange("(dc p) s -> p dc s", p=128))
        for w_sb, dw in ((wq_sb, dwq), (wk_sb, dwk), (wv_sb, dwv)):
            nc.scalar.dma_start(
                out=w_sb.rearrange("p (dc hd) -> p dc hd", hd=HD),
                in_=dw.rearrange("(dc p) hd -> p dc hd", p=128))
        nc.scalar.dma_start(out=wsums_sb, in_=dwsums)

        for tb in range(4):
            if tb == 0:
                q4 = qpool.tile([128, 4, DM], BF16, tag=f"q4_{tb % 2}")
                nc.sync.dma_start(
                    out=q4,
                    in_=dqbf[tb * 512:(tb + 1) * 512, :]
                        .rearrange("(a p) m -> p a m", p=128))
            else:
                q4 = q4_next
            if tb < 3:
                q4_next = qpool.tile([128, 4, DM], BF16, tag=f"q4_{(tb + 1) % 2}")
                nc.sync.dma_start(
                    out=q4_next,
                    in_=dqbf[(tb + 1) * 512:(tb + 2) * 512, :]
                        .rearrange("(a p) m -> p a m", p=128))

            negmu4 = spool.tile([128, 4], BF16, tag="negmu")
            rstd4 = spool.tile([128, 4], BF16, tag="rstdb")
            rstd4f = spool.tile([128, 4], F32, tag="rstdf")
            for k in range(4):
                st = spool.tile([128, 2, 6], F32, tag="st")
                nc.vector.bn_stats(out=st[:, 0, :], in_=q4[:, k, 0:512])
                nc.vector.bn_stats(out=st[:, 1, :], in_=q4[:, k, 512:1024])
                mv = spool.tile([128, 2], F32, tag="mv")
                nc.vector.bn_aggr(out=mv, in_=st)
                inv = spool.tile([128, 1], F32, tag="inv")
                nc.vector.tensor_scalar_add(inv, mv[:, 1:2], LN_EPS)
                nc.vector.reciprocal(out=inv, in_=inv)
                # rstd = sqrt(1/(var+eps)); sqrt set loads once, exp set later
                nc.scalar.sqrt(out=rstd4f[:, k:k + 1], in_=inv)
                nc.vector.tensor_scalar_mul(negmu4[:, k:k + 1], mv[:, 0:1], -1.0)
                nc.vector.tensor_copy(out=rstd4[:, k:k + 1], in_=rstd4f[:, k:k + 1])
                nc.vector.tensor_scalar_mul(rstd_sc[:, tb * 4 + k: tb * 4 + k + 1],
                                            rstd4f[:, k:k + 1], SCALE)
                negmr = spool.tile([128, 1], F32, tag="negmr")
                nc.vector.tensor_tensor(out=negmr, in0=negmu4[:, k:k + 1],
                                        in1=rstd4f[:, k:k + 1], op=Alu.mult)
                # qn residual (bf16, in place over q)
                nc.vector.tensor_scalar(out=q4[:, k, :], in0=q4[:, k, :],
                                        scalar1=rstd4f[:, k:k + 1], scalar2=negmr,
                                        op0=Alu.mult, op1=Alu.add)
                if gbc is not None:
                    nc.vector.tensor_tensor(out=q4[:, k, :], in0=q4[:, k, :],
                                            in1=gbc, op=Alu.mult)
                    nc.vector.tensor_tensor(out=q4[:, k, :], in0=q4[:, k, :],
                                            in1=bbc, op=Alu.add)
            nc.gpsimd.dma_start(
                out=dqn[tb * 512:(tb + 1) * 512, :]
                    .rearrange("(a p) m -> p a m", p=128),
                in_=q4)

            # stats -> DRAM -> (row for K=1 matmuls, partition-broadcast rstd)
            srow = rpool.tile([128, 8], BF16, tag="srow")
            nc.vector.tensor_copy(out=srow[:, 0:4], in_=negmu4)
            nc.vector.tensor_copy(out=srow[:, 4:8], in_=rstd4)
            nc.sync.dma_start(
                out=dstat[tb * 8:(tb + 1) * 8, :].rearrange("c p -> p c"),
                in_=srow)
            csl = slice(tb * 512, (tb + 1) * 512)
            rv0 = dstat[tb * 8:tb * 8 + 4, :].rearrange("k p -> (k p)")
            nc.sync.dma_start(
                out=nmrow[0:1, csl],
                in_=bass.AP(tensor=rv0.tensor, offset=rv0.offset,
                            ap=[[0, 1]] + list(rv0.ap)))
            rv = dstat[tb * 8 + 4:tb * 8 + 8, :].rearrange("k p -> (k p)")
            nc.sync.dma_start(
                out=rstd_bc[:, csl],
                in_=bass.AP(tensor=rv.tensor, offset=rv.offset,
                            ap=[[0, 128]] + list(rv.ap)))

            # ---- folded QKV for seq block sc == tb ----
            sc = tb
            ssl = slice(sc * 512, (sc + 1) * 512)
            for pi, (w_sb, dstT) in enumerate(((wq_sb, hqT), (wk_sb, hkT))):
                for hc in range(NHC):
                    ps = psQK.tile([128, 512], F32, tag="psqk")
                    for dc in range(NDC):
                        nc.tensor.matmul(
                            ps,
                            lhsT=w_sb[:, dc * HD + hc * 128: dc * HD + (hc + 1) * 128],
                            rhs=qT_sb[:, dc * SEQ + sc * 512: dc * SEQ + (sc + 1) * 512],
                            start=(dc == 0), stop=False)
                        # -mu fold: + wsum[hd] * (-mu[i])
                    nc.tensor.matmul(
                        ps, lhsT=wsums_sb[0:1, pi * HD + hc * 128: pi * HD + (hc + 1) * 128],
                        rhs=nmrow[0:1, ssl],
                        start=False, stop=trivial)
                    if not trivial:
                        nc.tensor.matmul(
                            ps, lhsT=wbeta_sb[0:1, pi * HD + hc * 128: pi * HD + (hc + 1) * 128],
                            rhs=ones_row[0:1, ssl],
                            start=False, stop=True)
                    osl = slice(hc * SEQ + sc * 512, hc * SEQ + (sc + 1) * 512)
                    if pi == 0:
                        # Q side: multiply rstd[i] during the PSUM->SBUF move
                        nc.vector.tensor_tensor(out=dstT[:, osl], in0=ps,
                                                in1=rstd_bc[:, ssl], op=Alu.mult)
                    else:
                        # K side: rstd[j] rides the exp's per-partition scale
                        nc.vector.tensor_copy(out=dstT[:, osl], in_=ps)
            for k in range(4):
                jc = 4 * tb + k
                jsl = slice(jc * 128, (jc + 1) * 128)
                ps = psQK.tile([128, HD], F32, tag="psv")
                for dc in range(NDC):
                    nc.tensor.matmul(
                        ps,
                        lhsT=qT_sb[:, dc * SEQ + jc * 128: dc * SEQ + jc * 128 + 128],
                        rhs=wv_sb[:, dc * HD:(dc + 1) * HD],
                        start=(dc == 0), stop=False)
                nc.tensor.matmul(
                    ps, lhsT=nmrow[0:1, jsl], rhs=wsums_sb[0:1, 2 * HD:3 * HD],
                    start=False, stop=trivial)
                if not trivial:
                    nc.tensor.matmul(
                        ps, lhsT=ones_row[0:1, jsl], rhs=wbeta_sb[0:1, 2 * HD:3 * HD],
                        start=False, stop=True)
                blk = hv[:, jc * HPC * HVW:(jc + 1) * HPC * HVW]
                blk3 = blk.rearrange("p (h x) -> p h x", x=HVW)
                nc.vector.tensor_scalar(
                    out=blk3[:, :, 0:DH],
                    in0=ps.rearrange("p (h x) -> p h x", x=DH),
                    scalar1=rstd4f[:, k:k + 1], scalar2=None, op0=Alu.mult)
        hv4 = hv.rearrange("p (j h x) -> p j h x", h=HPC, x=HVW)
        nc.vector.memset(hv4[:, :, :, DH:HVW], 1.0)

    # ---------------- Phase B: attention (two heads per pass) ---------------
    drecip = nc.dram_tensor(f"recip_scratch{nc.next_id()}", [HPC, SEQ], F32).ap()
    with tc.tile_pool(name="mk", bufs=1) as mkpool, \
         tc.tile_pool(name="pp", bufs=3) as ppool, \
         tc.tile_pool(name="stg", bufs=2) as stpool, \
         tc.tile_pool(name="den", bufs=1) as denpool, \
         tc.tile_pool(name="sps", bufs=3, space="PSUM") as spsum, \
         tc.tile_pool(name="vps", bufs=1, space="PSUM") as vpsum:
        # whole mask resident: [j, i] in jc column blocks, 8 chunked DMAs
        mask_all = mkpool.tile([128, NT * SEQ], BF16)
        for c in range(8):
            eng = nc.sync if c % 2 == 0 else nc.scalar
            eng.dma_start(
                out=mask_all[:, c * 2 * SEQ:(c + 1) * 2 * SEQ]
                    .rearrange("p (a i) -> p a i", i=SEQ),
                in_=dmask[c * 256:(c + 1) * 256, :]
                    .rearrange("(a p) i -> p a i", p=128))
        nc.scalar.dma_start(
            out=wo_sb.rearrange("p (hc m) -> p hc m", m=DM),
            in_=dwo.rearrange("(hc p) m -> p hc m", p=128))

        def scores(hp, ic, jc):
            # both heads' scores for a 512-wide i chunk: head a -> bank 0,
            # head b -> bank 1 of one [128, 1024] psum tile (row-tiled pair)
            sAB = spsum.tile([128, 1024], F32, tag="sAB")
            kslice = slice(hp * SEQ + jc * 128, hp * SEQ + (jc + 1) * 128)
            qsl = slice(hp * SEQ + ic * 512, hp * SEQ + (ic + 1) * 512)
            nc.tensor.matmul(sAB[:, 0:512],
                             lhsT=hkT[0:64, kslice], rhs=hqT[0:64, qsl],
                             start=True, stop=True)
            nc.tensor.matmul(sAB[:, 512:1024],
                             lhsT=hkT[64:128, kslice], rhs=hqT[64:128, qsl],
                             start=True, stop=True)
            return sAB

        for hp in range(HPC // 2):
            ha, hb = 2 * hp, 2 * hp + 1
            den_hp = denpool.tile([2, SEQ], F32, tag="den")
            for ic in range(4):
                vA = vpsum.tile([65, 512], F32, tag="vA")
                vB = vpsum.tile([65, 512], F32, tag="vB")
                sAB = scores(hp, ic, 0)
                for jc in range(NT):
                    pa = ppool.tile([128, 1024], BF16, tag="pa")
                    # exp over BOTH heads' scores; K-side rstd[j] via scale AP
                    nc.scalar.activation(out=pa, in_=sAB, func=Act.Exp,
                                         scale=rstd_sc[:, jc:jc + 1])
                    mk = mask_all[:, jc * SEQ + ic * 512: jc * SEQ + ic * 512 + 512]
                    nc.vector.tensor_mul(pa[:, 0:512], pa[:, 0:512], mk)
                    nc.vector.tensor_mul(pa[:, 512:1024], pa[:, 512:1024], mk)
                    # pipeline: next block's scores issue before this AV so
                    # the exp stream never stalls behind AV sem-waits
                    if jc + 1 < NT:
                        sAB = scores(hp, ic, jc + 1)
                    va_l = hv[:, jc * HPC * HVW + ha * HVW: jc * HPC * HVW + ha * HVW + HVW]
                    vb_l = hv[:, jc * HPC * HVW + hb * HVW: jc * HPC * HVW + hb * HVW + HVW]
                    nc.tensor.matmul(vA, lhsT=va_l, rhs=pa[:, 0:512],
                                     start=(jc == 0), stop=(jc == NT - 1))
                    nc.tensor.matmul(vB, lhsT=vb_l, rhs=pa[:, 512:1024],
                                     start=(jc == 0), stop=(jc == NT - 1))
                isl = slice(hp * SEQ + ic * 512, hp * SEQ + (ic + 1) * 512)
                dsl = slice(ic * 512, (ic + 1) * 512)
                # denominators (psum row 64) stage to SBUF (DMA cannot read
                # PSUM) then DMA to den_hp partitions 0/1; head-b vec rows
                # move psum 0:64 -> sbuf 64:128 -- cross-partition, DMA only.
                dsa = stpool.tile([65, 512], F32, tag="dsa")
                dsb = stpool.tile([65, 512], F32, tag="dsb")
                nc.vector.tensor_copy(out=dsa[64:65, :], in_=vA[64:65, :])
                nc.vector.tensor_copy(out=dsb[64:65, :], in_=vB[64:65, :])
                nc.sync.dma_start(out=den_hp[0:1, dsl], in_=dsa[64:65, :])
                nc.sync.dma_start(out=den_hp[1:2, dsl], in_=dsb[64:65, :])
                nc.vector.tensor_copy(out=vecT[0:64, isl], in_=vA[0:64, :])
                stage = stpool.tile([64, 512], BF16, tag="stg")
                nc.vector.tensor_copy(out=stage, in_=vB[0:64, :])
                nc.gpsimd.dma_start(out=vecT[64:128, isl], in_=stage)
            # normalize this head pair while the next pair's attention runs
            recip_hp = denpool.tile([2, SEQ], F32, tag="recip")
            nc.vector.reciprocal_approx_fast(out=recip_hp, in_=den_hp)
            nc.sync.dma_start(out=drecip[ha:hb + 1, :], in_=recip_hp)
            bc_hp = denpool.tile([128, SEQ], F32, tag="bc")
            for h, lo in ((ha, 0), (hb, 64)):
                row = drecip[h:h + 1, :]
                nc.sync.dma_start(
                    out=bc_hp[lo:lo + 64, :],
                    in_=bass.AP(tensor=row.tensor, offset=row.offset,
                                ap=[[0, 64]] + list(row.ap[1:])))
            hsl = slice(hp * SEQ, (hp + 1) * SEQ)
            nc.vector.tensor_tensor(out=vecT[:, hsl], in0=vecT[:, hsl],
                                    in1=bc_hp, op=Alu.mult)

    # ---------------- Phase C: O-projection ---------------------------------
    with tc.tile_pool(name="po", bufs=4, space="PSUM") as opool, \
         tc.tile_pool(name="outs", bufs=2) as outpool:
        for ob in range(NT // 4):
            outt = outpool.tile([128, 4, DM], F32, tag="outt")
            for k in range(4):
                it = ob * 4 + k
                for mc in range(2):
                    po = opool.tile([128, 512], F32, tag="po")
                    for hc in range(NHC):
                        nc.tensor.matmul(
                            po,
                            lhsT=vecT[:, hc * SEQ + it * 128: hc * SEQ + (it + 1) * 128],
                            rhs=wo_sb[:, hc * DM + mc * 512: hc * DM + (mc + 1) * 512],
                            start=(hc == 0), stop=(hc == NHC - 1))
                    nc.vector.tensor_copy(out=outt[:, k, mc * 512:(mc + 1) * 512],
                                          in_=po)
            nc.sync.dma_start(
                out=dpart[ob * 512:(ob + 1) * 512, :]
                    .rearrange("(a p) m -> p a m", p=128),
                in_=outt)


_NC_CACHE = {}


def _build(gamma_trivial, repeat=1):
    key = (bool(gamma_trivial), repeat)
    if key in _NC_CACHE:
        return _NC_CACHE[key]
    nc = bacc.Bacc("TRN2", target_bir_lowering=False, debug=False,
                   num_devices=NCORES)
    dqbf = nc.dram_tensor("qbf", [SEQ, DM], BF16, kind="ExternalInput").ap()
    dqt = nc.dram_tensor("qt", [DM, SEQ], BF16, kind="ExternalInput").ap()
    dmask = nc.dram_tensor("maskt", [SEQ, SEQ], BF16, kind="ExternalInput").ap()
    dwq = nc.dram_tensor("wq", [DM, HD], BF16, kind="ExternalInput").ap()
    dwk = nc.dram_tensor("wk", [DM, HD], BF16, kind="ExternalInput").ap()
    dwv = nc.dram_tensor("wv", [DM, HD], BF16, kind="ExternalInput").ap()
    dwo = nc.dram_tensor("wo", [HD, DM], BF16, kind="ExternalInput").ap()
    dwsums = nc.dram_tensor("wsums", [1, 3 * HD], BF16, kind="ExternalInput").ap()
    dwbeta = dgammabf = dbetabf = None
    if not gamma_trivial:
        dwbeta = nc.dram_tensor("wbeta", [1, 3 * HD], BF16, kind="ExternalInput").ap()
        dgammabf = nc.dram_tensor("gammabf", [DM], BF16, kind="ExternalInput").ap()
        dbetabf = nc.dram_tensor("betabf", [DM], BF16, kind="ExternalInput").ap()
    dqn = nc.dram_tensor("qn_out", [SEQ, DM], BF16, kind="ExternalOutput").ap()
    dpart = nc.dram_tensor("part_out", [SEQ, DM], F32, kind="ExternalOutput").ap()
    with tile.TileContext(nc) as tc:
        for _rep in range(repeat):
            with ExitStack() as ctx:
                _mha_tile(ctx, tc, dqbf, dqt, dmask, dwq, dwk, dwv, dwo,
                          dwsums, dwbeta, dgammabf, dbetabf, dqn, dpart)
    nc.compile()
    _NC_CACHE[key] = nc
    return nc


def _run(nc, in_maps, **kwargs):
    return run_bass_kernel_spmd(nc, in_maps, list(range(NCORES)), **kwargs)


def make_in_maps(q, attn_mask, Wq, Wk, Wv, Wo, gamma, beta, gamma_trivial):
    bf = ml_dtypes.bfloat16
    q32 = np.asarray(q, dtype=np.float32)
    maskt = np.ascontiguousarray(
        (~np.asarray(attn_mask, dtype=bool)).T.astype(bf))
    g = np.asarray(gamma, dtype=np.float32)
    b = np.asarray(beta, dtype=np.float32)
    Wq32 = np.asarray(Wq, dtype=np.float32) * g[:, None]
    Wk32 = np.asarray(Wk, dtype=np.float32) * g[:, None]
    Wv32 = np.asarray(Wv, dtype=np.float32) * g[:, None]
    Wo32 = np.asarray(Wo, dtype=np.float32)
    in_maps = []
    for c in range(NCORES):
        bi, grp = c // 2, c % 2
        cols = slice(grp * HD, (grp + 1) * HD)
        wqc = Wq32[:, cols]
        wkc = Wk32[:, cols]
        wvc = Wv32[:, cols]
        wsums = np.concatenate([wqc.sum(0), wkc.sum(0), wvc.sum(0)])
        qb = q32[bi]
        m = {
            "qbf": np.ascontiguousarray(qb.astype(bf)),
            "qt": np.ascontiguousarray(qb.T).astype(bf),
            "maskt": maskt,
            "wq": np.ascontiguousarray(wqc.astype(bf)),
            "wk": np.ascontiguousarray(wkc.astype(bf)),
            "wv": np.ascontiguousarray(wvc.astype(bf)),
            "wo": np.ascontiguousarray(Wo32[cols, :].astype(bf)),
            "wsums": np.ascontiguousarray(wsums[None, :].astype(bf)),
        }
        if not gamma_trivial:
            # beta enters the projections through the UNFOLDED weights
            wbeta = np.concatenate([
                b @ np.asarray(Wq, np.float32)[:, cols],
                b @ np.asarray(Wk, np.float32)[:, cols],
                b @ np.asarray(Wv, np.float32)[:, cols]])
            m["wbeta"] = np.ascontiguousarray(wbeta[None, :].astype(bf))
            m["gammabf"] = g.astype(bf)
            m["betabf"] = b.astype(bf)
        in_maps.append(m)
    return in_maps


def kernel(q, attn_mask, Wq, Wk, Wv, Wo, gamma, beta):
    gamma_np = np.asarray(gamma, dtype=np.float32)
    beta_np = np.asarray(beta, dtype=np.float32)
    gamma_trivial = bool(np.all(gamma_np == 1.0) and np.all(beta_np == 0.0))
    nc = _build(gamma_trivial)
    in_maps = make_in_maps(q, attn_mask, Wq, Wk, Wv, Wo, gamma_np, beta_np,
                           gamma_trivial)
    res = _run(nc, in_maps).results
    out = np.empty((BSZ, SEQ, DM), dtype=np.float32)
    for b in range(BSZ):
        out[b] = np.asarray(res[2 * b]["qn_out"]).astype(np.float32)
        out[b] += res[2 * b]["part_out"]
        out[b] += res[2 * b + 1]["part_out"]
    return out


if __name__ == "__main__":
    rng = np.random.default_rng(0)
    ins = {
        "q": rng.standard_normal((BSZ, SEQ, DM), dtype=np.float32),
        "attn_mask": rng.integers(0, 2, (SEQ, SEQ)).astype(bool),
        "Wq": rng.standard_normal((DM, NH * DH), dtype=np.float32) * 0.03,
        "Wk": rng.standard_normal((DM, NH * DH), dtype=np.float32) * 0.03,
        "Wv": rng.standard_normal((DM, NH * DH), dtype=np.float32) * 0.03,
        "Wo": rng.standard_normal((NH * DH, DM), dtype=np.float32) * 0.03,
        "gamma": np.ones(DM, np.float32),
        "beta": np.zeros(DM, np.float32),
    }
    out = kernel(**ins)
    print("kernel ran, out shape", out.shape, out.dtype)
